# revision 35
# baseline (speedup 1.0000x reference)
"""EFLSTM Trainium2 kernel: 8-core tensor-parallel LSTM + fused head.

Strategy (8 NeuronCores, SPMD single program, per-core data differs):
  - x ships T-sharded in fp8 (host->device staging is the dominant per-call
    cost); one on-device AllGather rebuilds the full x.
  - The input projection x @ W_ih.T + b is precomputed for ALL timesteps as
    one big GEMM (f=512 matmuls, 4 steps per tile) into xg_all in DRAM.
  - Tensor-parallel recurrence over the 4H gate dimension: core k owns gate
    rows [k*128:(k+1)*128] of each gate (i, f, g, o) and computes h rows
    [k*128:(k+1)*128] each step. A per-step AllGather rebuilds the full
    transposed hidden state h_T [H=1024, B=128] on every core.
  - Per step: one identity-seeded matmul loads xg_t into PSUM, 32 recurrent
    matmuls accumulate on top, gate nonlinearities + cell update produce the
    h slice, which is exchanged via AllGather into an 8-slot SBUF ring.
  - The FC head (one 128-wide slice of C per core; cores k and k+4 duplicate
    a slice, fc2 pre-scaled by 0.5) is evaluated every 4 steps with f=512
    matmuls off the h ring; partials accumulate in SBUF and are stored once.
  - Tail: one ReduceScatter sums the fc2 partials; log_softmax on each
    core's token shard produces the output.
"""

import numpy as np
import ml_dtypes

import concourse.bacc as bacc
import concourse.mybir as mybir
import concourse.tile as tile
from concourse.bass_utils import run_bass_kernel_spmd

F32 = mybir.dt.float32
BF16 = mybir.dt.bfloat16
FP8 = mybir.dt.float8e4
AF = mybir.ActivationFunctionType
ALU = mybir.AluOpType

N_CORES = 8
B = 128
T = 512
DIMS = (300, 74, 35)
D = sum(DIMS)  # 409
DP = D + 1     # 410: extra ones-row folds the gate bias into the matmul
H = 1024
G = 4 * H
C = 512
O = 7
HSL = H // N_CORES        # 128 h rows per core
GSL = 4 * HSL             # 512 gate rows per core
KC_X = [128, 128, 128, DP - 3 * 128]   # contraction chunks over D+1
NKX = len(KC_X)
NKH = H // 128            # 8 contraction chunks over H
TOK = B * T

import os
NO_CC = bool(int(os.environ.get("EF_NO_CC", "0")))
# GUT=6: empty main loop + no precompute (infra-floor measurement)
GUT = int(os.environ.get("EF_GUT", "0"))
# comma-separated per-component cuts (timing-only): ew,rec,exch,xg,head
CUT = set(x for x in os.environ.get("EF_CUT", "").split(",") if x)

# weight blob layout (bf16, p-major)
WCOLS = NKX * GSL + NKH * GSL + NKH * 128 + O + 128   # + eye(128)
O_WIH = 0
O_WHH = O_WIH + NKX * GSL
O_FC1 = O_WHH + NKH * GSL
O_FC2 = O_FC1 + NKH * 128
O_EYE = O_FC2 + O


def build_kernel(t_steps=T):
    assert t_steps % 32 == 0
    nc = bacc.Bacc("TRN2", target_bir_lowering=False, debug=False,
                   num_devices=N_CORES)

    t_sh4 = t_steps // N_CORES // 4   # t4-blocks per core shard
    nt4 = t_steps // 4
    xTs = nc.dram_tensor("xTs", [t_sh4, 128, NKX, 4, B], FP8,
                         kind="ExternalInput")
    wblob = nc.dram_tensor("wblob", [128, WCOLS], BF16, kind="ExternalInput")
    fblob = nc.dram_tensor("fblob", [128, 1 + O], F32, kind="ExternalInput")

    n_tok_loc = B * t_steps // N_CORES
    out_sh = nc.dram_tensor("out_sh", [n_tok_loc, O], F32, kind="ExternalOutput")

    with tile.TileContext(nc) as tc:
        with (
            tc.tile_pool(name="const", bufs=1) as const,
            tc.tile_pool(name="xq", bufs=3) as xqp,
            tc.tile_pool(name="xg4", bufs=3) as xg4p,
            tc.tile_pool(name="xg", bufs=4) as xgp,
            tc.tile_pool(name="pg", bufs=2, space="PSUM") as pgp,
            tc.tile_pool(name="ph", bufs=2, space="PSUM") as php,
            tc.tile_pool(name="pf", bufs=2, space="PSUM") as pfp,
            tc.tile_pool(name="ew", bufs=2) as ewp,
            tc.tile_pool(name="dram", bufs=2, space="DRAM") as dramp,
            tc.tile_pool(name="dram1", bufs=1, space="DRAM") as dramp1,
        ):
            # ---- resident weights: one contiguous blob load, sliced views --
            w_sb = const.tile([128, WCOLS], BF16)
            nc.sync.dma_start(w_sb[:], wblob[:])
            wih_sb = w_sb[:, O_WIH:O_WHH].rearrange("p (k g) -> p k g", k=NKX)
            whh_sb = w_sb[:, O_WHH:O_FC1].rearrange("p (k g) -> p k g", k=NKH)
            fc1_sb = w_sb[:, O_FC1:O_FC2].rearrange("p (k c) -> p k c", k=NKH)
            fc2w_sb = w_sb[:, O_FC2:O_FC2 + O]
            eye_sb = w_sb[:, O_EYE:O_EYE + 128]
            f_sb = const.tile([128, 1 + O], F32)
            nc.sync.dma_start(f_sb[:], fblob[:])
            fc1b_sb = f_sb[:, 0:1]
            fc2b_sb = f_sb[:, 1:1 + O]

            c_sb = const.tile([128, 128], F32)   # persistent cell state slice
            # head partials accumulate in SBUF; one bulk store after the loop
            out_acc = const.tile([128, t_steps, O], F32)
            # gathered h_T ring: 8 steps deep, [p, slot, kc, b]
            hT8 = const.tile([128, 8, NKH, B], BF16)

            out_part = dramp1.tile([n_tok_loc * N_CORES, O], F32)

            # ---- rebuild full xT on-device: copy shard to an internal DRAM
            # tile (collectives cannot touch kernel I/O), then AllGather ----
            x_stage = dramp1.tile([t_sh4 * 128, NKX * 4 * B], FP8)
            nc.sync.dma_start(
                x_stage[:], xTs[:].rearrange("t p k q b -> (t p) (k q b)"))
            xT_full = dramp1.tile([nt4 * 128, NKX * 4 * B], FP8,
                                  addr_space="Shared")
            nc.gpsimd.collective_compute(
                "AllGather",
                ALU.bypass,
                replica_groups=[list(range(N_CORES))],
                ins=[x_stage[:].opt()],
                outs=[xT_full[:].opt()],
            )
            xT4 = xT_full[:].rearrange("(t p) n -> t p n", p=128).rearrange(
                "t p (k q b) -> t p k q b", k=NKX, q=4)

            gate_slices = [(mc * 128, (mc + 1) * 128) for mc in range(4)]

            if GUT == 6 or "head" in CUT:
                nc.vector.memset(out_acc[:], 0.0)
            if "exch" in CUT:
                nc.vector.memset(hT8[:], 0.0)
            xg0 = None
            if "xg" in CUT:
                xg0 = const.tile([128, GSL], BF16)
                nc.vector.memset(xg0[:], 0.0)

            # ---- precompute xg_all[t] = x_t @ W_ih.T + b for all steps ----
            xg_all = None
            if GUT == 0:
                xg_all = dramp1.tile([t_steps, 128, GSL], BF16)
                for q in range(nt4):
                    xts4 = xqp.tile([128, NKX, 4, B], BF16)
                    nc.gpsimd.dma_start(xts4[:], xT4[q])   # fp8 -> bf16
                    xg4 = xg4p.tile([128, 4, 4, B], BF16)  # [p, t', mc, b]
                    for mc in range(4):
                        lo, hi = gate_slices[mc]
                        pxg = pgp.tile([128, 4 * B], F32, tag="pg")
                        for kc in range(NKX):
                            kk = KC_X[kc]
                            nc.tensor.matmul(
                                pxg[:],
                                wih_sb[:kk, kc, lo:hi],
                                xts4[:kk, kc, :, :],
                                start=(kc == 0),
                                stop=(kc == NKX - 1),
                            )
                        nc.vector.tensor_copy(
                            xg4[:, :, mc, :],
                            pxg[:].rearrange("p (q b) -> p q b", q=4))
                    nc.sync.dma_start(
                        xg_all[q * 4:(q + 1) * 4].rearrange("t p n -> p t n"),
                        xg4[:].rearrange("p q m b -> p q (m b)"))

            # ---- recurrence ----
            # batch is processed as two 64-wide halves with interleaved
            # exchanges: half A's AllGather latency hides under half B's
            # compute and vice versa.  pg free-axis layout stays
            # (mc*128 + h*64 + b'); per-(half, gate) slices are 64 wide.
            BH = B // 2
            for t in range(t_steps if GUT == 0 else 0):
                pg = pgp.tile([128, GSL], F32, tag="pg")
                xg_sb = None
                if "xg" not in CUT:
                    xg_sb = xgp.tile([128, GSL], BF16)
                    nc.sync.dma_start(xg_sb[:], xg_all[t])
                for hf in range(2):
                    bl, bh = hf * BH, (hf + 1) * BH
                    # seed xg into PSUM (identity matmul, f=64 per gate)
                    for mc in range(4):
                        lo = mc * 128 + hf * BH
                        if "xg" not in CUT:
                            nc.tensor.matmul(
                                pg[:, lo:lo + BH], eye_sb,
                                xg_sb[:, lo:lo + BH],
                                start=True, stop=(t == 0 or "rec" in CUT))
                        elif t == 0:
                            nc.tensor.matmul(
                                pg[:, lo:lo + BH], eye_sb, xg0[:, lo:lo + BH],
                                start=True, stop=True)
                    # recurrent matmuls (consume this half's gathered h)
                    if t > 0 and "rec" not in CUT:
                        sl = (t - 1) % 8
                        for mc in range(4):
                            lo = mc * 128 + hf * BH
                            gl = mc * 128
                            for kc in range(NKH):
                                nc.tensor.matmul(
                                    pg[:, lo:lo + BH],
                                    whh_sb[:, kc, gl:gl + 128],
                                    hT8[:, sl, kc, bl:bh],
                                    start=("xg" in CUT and kc == 0),
                                    stop=(kc == NKH - 1),
                                )

                    if "ew" not in CUT:
                        # elementwise: gates -> h half-slice
                        gnl = ewp.tile([128, 4, BH], F32, tag="gnl")
                        # i, f gates (cols mc0/mc1 of this half)
                        nc.scalar.activation(
                            gnl[:, 0, :], pg[:, hf * BH:hf * BH + BH],
                            AF.Sigmoid)
                        nc.scalar.activation(
                            gnl[:, 1, :], pg[:, 128 + hf * BH:128 + hf * BH + BH],
                            AF.Sigmoid)
                        if t > 0:
                            fcs = ewp.tile([128, BH], F32, tag="fcs")
                            nc.vector.tensor_mul(fcs[:], gnl[:, 1, :],
                                                 c_sb[:, bl:bh])
                        nc.scalar.activation(
                            gnl[:, 2, :], pg[:, 256 + hf * BH:256 + hf * BH + BH],
                            AF.Tanh)
                        nc.scalar.activation(
                            gnl[:, 3, :], pg[:, 384 + hf * BH:384 + hf * BH + BH],
                            AF.Sigmoid)
                        ig = ewp.tile([128, BH], F32, tag="ig")
                        nc.vector.tensor_mul(ig[:], gnl[:, 0, :], gnl[:, 2, :])
                        if t == 0:
                            nc.vector.tensor_copy(c_sb[:, bl:bh], ig[:])
                        else:
                            nc.vector.tensor_add(c_sb[:, bl:bh], fcs[:], ig[:])
                        tc_t = ewp.tile([128, BH], F32, tag="tct")
                        nc.scalar.activation(tc_t[:], c_sb[:, bl:bh], AF.Tanh)
                        h_sl = ewp.tile([128, BH], BF16, tag="hsl")
                        nc.vector.tensor_mul(h_sl[:], gnl[:, 3, :], tc_t[:])
                    else:
                        h_sl = ewp.tile([128, BH], BF16, tag="hsl")
                        nc.vector.tensor_copy(h_sl[:], pg[:, bl:bh])

                    if "exch" not in CUT:
                        # exchange: AllGather this half's h_T across 8 cores
                        bnc_in = dramp.tile([128, BH], BF16, tag=f"bi{hf}")
                        nc.sync.dma_start(bnc_in[:], h_sl[:])
                        bnc_out = dramp.tile(
                            [H, BH], BF16, tag=f"bo{hf}",
                            addr_space="Local" if NO_CC else "Shared")
                        if NO_CC:
                            for kc in range(NKH):
                                nc.gpsimd.dma_start(
                                    bnc_out[kc * 128:(kc + 1) * 128, :],
                                    bnc_in[:])
                        else:
                            nc.gpsimd.collective_compute(
                                "AllGather",
                                ALU.bypass,
                                replica_groups=[list(range(N_CORES))],
                                ins=[bnc_in[:].opt()],
                                outs=[bnc_out[:].opt()],
                            )
                        sl = t % 8
                        nc.sync.dma_start(
                            hT8[:, sl, 0:4, bl:bh],
                            bnc_out[0:512, :].rearrange(
                                "(k p) b -> p k b", p=128))
                        nc.sync.dma_start(
                            hT8[:, sl, 4:8, bl:bh],
                            bnc_out[512:1024, :].rearrange(
                                "(k p) b -> p k b", p=128))

                # head for steps t-4..t-1, every 4 steps (f=512 matmuls)
                if t >= 4 and t % 4 == 0 and "head" not in CUT:
                    emit_head4(nc, php, pfp, ewp, fc1_sb, fc1b_sb, fc2w_sb,
                               hT8, out_acc, t - 4)

            if GUT == 0 and "head" not in CUT:
                # head for the final 4 steps
                emit_head4(nc, php, pfp, ewp, fc1_sb, fc1b_sb, fc2w_sb,
                           hT8, out_acc, t_steps - 4)

            # bulk store of the accumulated head partials (token-major)
            nc.sync.dma_start(
                out_part[:].rearrange("(b t) o -> b t o", t=t_steps),
                out_acc[:])

            # ---- tail: ReduceScatter fc2 partials, bias + log_softmax ----
            rs_out = dramp1.tile([n_tok_loc, O], F32)
            nc.gpsimd.collective_compute(
                "ReduceScatter",
                ALU.add,
                replica_groups=[list(range(N_CORES))],
                ins=[out_part[:].opt()],
                outs=[rs_out[:].opt()],
            )
            # token rows viewed as [128, CH, O]: partition p owns tokens
            # p*CH..(p+1)*CH-1 — contiguous per-partition DMA lines
            CH = n_tok_loc // 128
            z_sb = ewp.tile([128, CH, O], F32)
            nc.sync.dma_start(
                z_sb[:], rs_out[:].rearrange("(p c) o -> p c o", p=128))
            res_sb = ewp.tile([128, CH, O], F32)
            for ch in range(CH):
                zb = ewp.tile([128, O], F32)
                nc.vector.tensor_add(zb[:], z_sb[:, ch, :], fc2b_sb)
                mx = ewp.tile([128, 1], F32)
                nc.vector.reduce_max(mx[:], zb[:], axis=mybir.AxisListType.X)
                sh = ewp.tile([128, O], F32)
                nc.vector.tensor_scalar_sub(sh[:], zb[:], mx[:])
                ex = ewp.tile([128, O], F32)
                nc.scalar.activation(ex[:], sh[:], AF.Exp)
                sm = ewp.tile([128, 1], F32)
                nc.vector.reduce_sum(sm[:], ex[:], axis=mybir.AxisListType.X)
                lg = ewp.tile([128, 1], F32)
                nc.scalar.activation(lg[:], sm[:], AF.Ln)
                nc.vector.tensor_scalar_sub(res_sb[:, ch, :], sh[:], lg[:])
            nc.sync.dma_start(
                out_sh[:].rearrange("(p c) o -> p c o", p=128), res_sb[:])

    nc.compile()
    return nc


def emit_head4(nc, php, pfp, ewp, fc1_sb, fc1b_sb, fc2w_sb, hT8, out_acc, t0):
    """fc1 + relu + fc2 partials for steps t0..t0+3 (slots contiguous in the
    8-ring because t0 % 4 == 0)."""
    s0 = t0 % 8
    ph = php.tile([128, 4 * B], F32)
    for kc in range(NKH):
        nc.tensor.matmul(ph[:], fc1_sb[:, kc, :], hT8[:, s0:s0 + 4, kc, :],
                         start=(kc == 0), stop=(kc == NKH - 1))
    hid = ewp.tile([128, 4, B], BF16)
    nc.scalar.activation(hid[:], ph[:].rearrange("p (q b) -> p q b", q=4),
                         AF.Relu, bias=fc1b_sb)
    pf = pfp.tile([128, 4 * O], F32)
    for s in range(4):
        nc.tensor.matmul(pf[:, s * O:(s + 1) * O], hid[:, s, :], fc2w_sb,
                         start=True, stop=True)
    nc.vector.tensor_copy(
        out_acc[:, t0:t0 + 4, :],
        pf[:].rearrange("p (q o) -> p q o", q=4))


_CACHED = {}


def _get_kernel(t_steps):
    if t_steps not in _CACHED:
        _CACHED[t_steps] = build_kernel(t_steps)
    return _CACHED[t_steps]


def prep_inputs(m_text, m_audio, m_video, W_ih, W_hh, b_ih, b_hh,
                fc1_w, fc1_b, fc2_w, fc2_b, t_steps=T):
    """Host-side layout prep; returns per-core input maps."""
    bf = ml_dtypes.bfloat16
    x = np.concatenate([np.asarray(m_text), np.asarray(m_audio),
                        np.asarray(m_video)], axis=-1).astype(np.float32)
    b_, t_, d_ = x.shape
    assert (b_, d_) == (B, D) and t_ == t_steps
    # x_T: [T, D+1(ones) padded to 4*128, B], stored [t4, p, k, t', b] so the
    # precompute load is one contiguous 2KB line per partition
    xTf = np.zeros((t_steps, NKX * 128, B), np.float32)
    xTf[:, :D, :] = x.transpose(1, 2, 0)
    xTf[:, D, :] = 1.0
    xTf = np.ascontiguousarray(
        xTf.reshape(t_steps // 4, 4, NKX, 128, B).transpose(
            0, 3, 2, 1, 4)).astype(ml_dtypes.float8_e4m3)

    W_ih = np.asarray(W_ih, np.float32)
    W_hh = np.asarray(W_hh, np.float32)
    bias = (np.asarray(b_ih) + np.asarray(b_hh)).astype(np.float32)
    fc1_w = np.asarray(fc1_w, np.float32)
    fc1_b = np.asarray(fc1_b, np.float32)
    fc2_w = np.asarray(fc2_w, np.float32)
    fc2_b = np.asarray(fc2_b, np.float32)

    in_maps = []
    gate_order = (0, 1, 2, 3)  # i, f, g, o (pytorch row-block order)
    eye = np.eye(128, dtype=np.float32)
    for k in range(N_CORES):
        rows = np.concatenate(
            [np.arange(g * H + k * HSL, g * H + (k + 1) * HSL)
             for g in gate_order])
        wih_sl = W_ih[rows, :]            # [512, 409]
        whh_sl = W_hh[rows, :]            # [512, 1024]
        b_sl = bias[rows]                 # [512]
        wihT_k = np.zeros((NKX * 128, GSL), np.float32)
        wihT_k[:D, :] = wih_sl.T
        wihT_k[D, :] = b_sl
        # p-major: [128, NKX*GSL]
        wih_pm = wihT_k.reshape(NKX, 128, GSL).transpose(1, 0, 2).reshape(
            128, NKX * GSL)
        whh_pm = whh_sl.T.reshape(NKH, 128, GSL).transpose(1, 0, 2).reshape(
            128, NKH * GSL)

        cc = k % 4                        # C chunk (cores k and k+4 duplicate)
        crows = np.arange(cc * 128, (cc + 1) * 128)
        fc1_pm = fc1_w[crows, :].T.reshape(NKH, 128, 128).transpose(
            1, 0, 2).reshape(128, NKH * 128)
        fc2wT_k = 0.5 * fc2_w[:, crows].T                    # [128, O]
        wblob_k = np.ascontiguousarray(np.concatenate(
            [wih_pm, whh_pm, fc1_pm, fc2wT_k, eye], axis=1)).astype(bf)

        fc1b_k = fc1_b[crows].reshape(128, 1)
        fc2b_k = np.broadcast_to(fc2_b[None, :], (128, O))
        fblob_k = np.ascontiguousarray(np.concatenate(
            [fc1b_k, fc2b_k], axis=1)).astype(np.float32)

        t_sh4 = t_steps // N_CORES // 4
        in_maps.append({
            "xTs": xTf[k * t_sh4:(k + 1) * t_sh4],
            "wblob": wblob_k,
            "fblob": fblob_k,
        })
    return in_maps


def run(inputs, t_steps=T, trace=False):
    nc = _get_kernel(t_steps)
    in_maps = prep_inputs(
        inputs["m_text"], inputs["m_audio"], inputs["m_video"],
        inputs["W_ih"], inputs["W_hh"], inputs["b_ih"], inputs["b_hh"],
        inputs["fc1_w"], inputs["fc1_b"], inputs["fc2_w"], inputs["fc2_b"],
        t_steps=t_steps)
    res = run_bass_kernel_spmd(
        nc, in_maps, core_ids=list(range(N_CORES)), trace=trace)
    shards = [res.results[k]["out_sh"] for k in range(N_CORES)]
    full = np.concatenate(shards, axis=0)          # [(b t), O] token-major
    out = full.reshape(B, t_steps, O)
    return out, res


def kernel(**inputs) -> np.ndarray:
    t_steps = np.asarray(inputs["m_text"]).shape[1]
    out, _ = run(inputs, t_steps=t_steps)
    return out.astype(np.float32)


# revision 38
# speedup vs baseline: 1.0308x; 1.0308x over previous
"""EFLSTM Trainium2 kernel: 8-core tensor-parallel LSTM + fused head.

Strategy (8 NeuronCores, SPMD single program, per-core data differs):
  - x ships T-sharded in fp8 (host->device staging is the dominant per-call
    cost); one on-device AllGather rebuilds the full x.
  - The input projection x @ W_ih.T + b is precomputed for ALL timesteps as
    one big GEMM (f=512 matmuls, 4 steps per tile) into xg_all in DRAM.
  - Tensor-parallel recurrence over the 4H gate dimension: core k owns gate
    rows [k*128:(k+1)*128] of each gate (i, f, g, o) and computes h rows
    [k*128:(k+1)*128] each step. A per-step AllGather rebuilds the full
    transposed hidden state h_T [H=1024, B=128] on every core.
  - Per step: one identity-seeded matmul loads xg_t into PSUM, 32 recurrent
    matmuls accumulate on top, gate nonlinearities + cell update produce the
    h slice, which is exchanged via AllGather into an 8-slot SBUF ring.
  - The FC head (one 128-wide slice of C per core; cores k and k+4 duplicate
    a slice, fc2 pre-scaled by 0.5) is evaluated every 4 steps with f=512
    matmuls off the h ring; partials accumulate in SBUF and are stored once.
  - Tail: one ReduceScatter sums the fc2 partials; log_softmax on each
    core's token shard produces the output.
"""

import numpy as np
import ml_dtypes

import concourse.bacc as bacc
import concourse.mybir as mybir
import concourse.tile as tile
from concourse.bass_utils import run_bass_kernel_spmd

F32 = mybir.dt.float32
BF16 = mybir.dt.bfloat16
FP8 = mybir.dt.float8e4
AF = mybir.ActivationFunctionType
ALU = mybir.AluOpType

N_CORES = 8
B = 128
T = 512
DIMS = (300, 74, 35)
D = sum(DIMS)  # 409
DP = D + 1     # 410: extra ones-row folds the gate bias into the matmul
H = 1024
G = 4 * H
C = 512
O = 7
HSL = H // N_CORES        # 128 h rows per core
GSL = 4 * HSL             # 512 gate rows per core
KC_X = [128, 128, 128, DP - 3 * 128]   # contraction chunks over D+1
NKX = len(KC_X)
NKH = H // 128            # 8 contraction chunks over H
TOK = B * T

import os
NO_CC = bool(int(os.environ.get("EF_NO_CC", "0")))
# GUT=6: empty main loop + no precompute (infra-floor measurement)
GUT = int(os.environ.get("EF_GUT", "0"))
# comma-separated per-component cuts (timing-only): ew,rec,exch,xg,head
CUT = set(x for x in os.environ.get("EF_CUT", "").split(",") if x)

# weight blob layout (bf16, p-major)
WCOLS = NKX * GSL + NKH * GSL + NKH * 128 + O + 128   # + eye(128)
O_WIH = 0
O_WHH = O_WIH + NKX * GSL
O_FC1 = O_WHH + NKH * GSL
O_FC2 = O_FC1 + NKH * 128
O_EYE = O_FC2 + O


def build_kernel(t_steps=T):
    assert t_steps % 32 == 0
    nc = bacc.Bacc("TRN2", target_bir_lowering=False, debug=False,
                   num_devices=N_CORES)

    t_sh4 = t_steps // N_CORES // 4   # t4-blocks per core shard
    nt4 = t_steps // 4
    xTs = nc.dram_tensor("xTs", [t_sh4, 128, NKX, 4, B], FP8,
                         kind="ExternalInput")
    wblob = nc.dram_tensor("wblob", [128, WCOLS], BF16, kind="ExternalInput")
    fblob = nc.dram_tensor("fblob", [128, 1 + O], F32, kind="ExternalInput")

    n_tok_loc = B * t_steps // N_CORES
    out_sh = nc.dram_tensor("out_sh", [n_tok_loc, O], F32, kind="ExternalOutput")

    with tile.TileContext(nc) as tc:
        with (
            tc.tile_pool(name="const", bufs=1) as const,
            tc.tile_pool(name="xq", bufs=3) as xqp,
            tc.tile_pool(name="xg4", bufs=3) as xg4p,
            tc.tile_pool(name="xg", bufs=4) as xgp,
            tc.tile_pool(name="pg", bufs=2, space="PSUM") as pgp,
            tc.tile_pool(name="ph", bufs=2, space="PSUM") as php,
            tc.tile_pool(name="pf", bufs=2, space="PSUM") as pfp,
            tc.tile_pool(name="ew", bufs=2) as ewp,
            tc.tile_pool(name="dram", bufs=2, space="DRAM") as dramp,
            tc.tile_pool(name="dram1", bufs=1, space="DRAM") as dramp1,
        ):
            # ---- resident weights: one contiguous blob load, sliced views --
            w_sb = const.tile([128, WCOLS], BF16)
            nc.sync.dma_start(w_sb[:], wblob[:])
            wih_sb = w_sb[:, O_WIH:O_WHH].rearrange("p (k g) -> p k g", k=NKX)
            whh_sb = w_sb[:, O_WHH:O_FC1].rearrange("p (k g) -> p k g", k=NKH)
            fc1_sb = w_sb[:, O_FC1:O_FC2].rearrange("p (k c) -> p k c", k=NKH)
            fc2w_sb = w_sb[:, O_FC2:O_FC2 + O]
            eye_sb = w_sb[:, O_EYE:O_EYE + 128]
            f_sb = const.tile([128, 1 + O], F32)
            nc.sync.dma_start(f_sb[:], fblob[:])
            fc1b_sb = f_sb[:, 0:1]
            fc2b_sb = f_sb[:, 1:1 + O]

            c_sb = const.tile([128, 128], F32)   # persistent cell state slice
            # head partials accumulate in SBUF; one bulk store after the loop
            out_acc = const.tile([128, t_steps, O], F32)
            # gathered h_T ring: 8 steps deep, [p, slot, kc, b]
            hT8 = const.tile([128, 8, NKH, B], BF16)

            out_part = dramp1.tile([n_tok_loc * N_CORES, O], F32)

            # ---- rebuild full xT on-device: copy shard to an internal DRAM
            # tile (collectives cannot touch kernel I/O), then AllGather ----
            x_stage = dramp1.tile([t_sh4 * 128, NKX * 4 * B], FP8)
            nc.sync.dma_start(
                x_stage[:], xTs[:].rearrange("t p k q b -> (t p) (k q b)"))
            xT_full = dramp1.tile([nt4 * 128, NKX * 4 * B], FP8,
                                  addr_space="Shared")
            nc.gpsimd.collective_compute(
                "AllGather",
                ALU.bypass,
                replica_groups=[list(range(N_CORES))],
                ins=[x_stage[:].opt()],
                outs=[xT_full[:].opt()],
            )
            xT4 = xT_full[:].rearrange("(t p) n -> t p n", p=128).rearrange(
                "t p (k q b) -> t p k q b", k=NKX, q=4)

            gate_slices = [(mc * 128, (mc + 1) * 128) for mc in range(4)]

            if GUT == 6 or "head" in CUT:
                nc.vector.memset(out_acc[:], 0.0)
            if "exch" in CUT:
                nc.vector.memset(hT8[:], 0.0)
            xg0 = None
            if "xg" in CUT:
                xg0 = const.tile([128, GSL], BF16)
                nc.vector.memset(xg0[:], 0.0)

            # ---- precompute xg_all[t] = x_t @ W_ih.T + b for all steps ----
            xg_all = None
            if GUT == 0:
                xg_all = dramp1.tile([t_steps, 128, GSL], BF16)
                for q in range(nt4):
                    xts4 = xqp.tile([128, NKX, 4, B], BF16)
                    nc.gpsimd.dma_start(xts4[:], xT4[q])   # fp8 -> bf16
                    xg4 = xg4p.tile([128, 4, 4, B], BF16)  # [p, t', mc, b]
                    for mc in range(4):
                        lo, hi = gate_slices[mc]
                        pxg = pgp.tile([128, 4 * B], F32, tag="pg")
                        for kc in range(NKX):
                            kk = KC_X[kc]
                            nc.tensor.matmul(
                                pxg[:],
                                wih_sb[:kk, kc, lo:hi],
                                xts4[:kk, kc, :, :],
                                start=(kc == 0),
                                stop=(kc == NKX - 1),
                            )
                        nc.vector.tensor_copy(
                            xg4[:, :, mc, :],
                            pxg[:].rearrange("p (q b) -> p q b", q=4))
                    nc.sync.dma_start(
                        xg_all[q * 4:(q + 1) * 4].rearrange("t p n -> p t n"),
                        xg4[:].rearrange("p q m b -> p q (m b)"))

            # ---- recurrence ----
            for t in range(t_steps if GUT == 0 else 0):
                # xg_t -> PSUM via identity-seeded matmul
                pg = pgp.tile([128, GSL], F32, tag="pg")
                if "xg" not in CUT:
                    xg_sb = xgp.tile([128, GSL], BF16)
                    nc.sync.dma_start(xg_sb[:], xg_all[t])
                    for mc in range(4):
                        lo, hi = gate_slices[mc]
                        nc.tensor.matmul(
                            pg[:, lo:hi], eye_sb, xg_sb[:, lo:hi],
                            start=True, stop=(t == 0 or "rec" in CUT))
                elif t == 0:
                    for mc in range(4):
                        lo, hi = gate_slices[mc]
                        nc.tensor.matmul(pg[:, lo:hi], eye_sb, xg0[:, lo:hi],
                                         start=True, stop=True)
                # recurrent matmuls (consume previous gathered h)
                if t > 0 and "rec" not in CUT:
                    sl = (t - 1) % 8
                    for mc in range(4):
                        lo, hi = gate_slices[mc]
                        for kc in range(NKH):
                            nc.tensor.matmul(
                                pg[:, lo:hi],
                                whh_sb[:, kc, lo:hi],
                                hT8[:, sl, kc, :],
                                start=("xg" in CUT and kc == 0),
                                stop=(kc == NKH - 1),
                            )

                # head for steps t-4..t-1, every 4 steps (f=512 matmuls)
                if t >= 4 and t % 4 == 0 and "head" not in CUT:
                    emit_head4(nc, php, pfp, ewp, fc1_sb, fc1b_sb, fc2w_sb,
                               hT8, out_acc, t - 4)

                if "ew" not in CUT:
                    # elementwise: gates -> h slice
                    gnl = ewp.tile([128, GSL], F32)
                    nc.scalar.activation(gnl[:, 0:256], pg[:, 0:256],
                                         AF.Sigmoid)
                    if t > 0:
                        fcs = ewp.tile([128, 128], F32)
                        nc.vector.tensor_mul(fcs[:], gnl[:, 128:256], c_sb[:])
                    nc.scalar.activation(gnl[:, 256:384], pg[:, 256:384],
                                         AF.Tanh)
                    nc.scalar.activation(gnl[:, 384:512], pg[:, 384:512],
                                         AF.Sigmoid)
                    ig = ewp.tile([128, 128], F32)
                    nc.vector.tensor_mul(ig[:], gnl[:, 0:128],
                                         gnl[:, 256:384])
                    if t == 0:
                        nc.vector.tensor_copy(c_sb[:], ig[:])
                    else:
                        nc.vector.tensor_add(c_sb[:], fcs[:], ig[:])
                    tc_t = ewp.tile([128, 128], F32)
                    nc.scalar.activation(tc_t[:], c_sb[:], AF.Tanh)
                    h_sl = ewp.tile([128, 128], BF16)
                    nc.vector.tensor_mul(h_sl[:], gnl[:, 384:512], tc_t[:])
                else:
                    h_sl = ewp.tile([128, 128], BF16)
                    nc.vector.tensor_copy(h_sl[:], pg[:, 0:128])

                if "exch" not in CUT:
                    # exchange: AllGather h_T across the 8 cores
                    bnc_in = dramp.tile([128, B], BF16)
                    nc.sync.dma_start(bnc_in[:], h_sl[:])
                    bnc_out = dramp.tile(
                        [H, B], BF16,
                        addr_space="Local" if NO_CC else "Shared")
                    if NO_CC:
                        for kc in range(NKH):
                            nc.gpsimd.dma_start(
                                bnc_out[kc * 128:(kc + 1) * 128, :],
                                bnc_in[:])
                    else:
                        nc.gpsimd.collective_compute(
                            "AllGather",
                            ALU.bypass,
                            replica_groups=[list(range(N_CORES))],
                            ins=[bnc_in[:].opt()],
                            outs=[bnc_out[:].opt()],
                        )
                    # two half-loads so the next step's rec starts at half
                    sl = t % 8
                    nc.sync.dma_start(
                        hT8[:, sl, 0:4, :],
                        bnc_out[0:512, :].rearrange(
                            "(k p) b -> p k b", p=128))
                    nc.sync.dma_start(
                        hT8[:, sl, 4:8, :],
                        bnc_out[512:1024, :].rearrange(
                            "(k p) b -> p k b", p=128))

            if GUT == 0 and "head" not in CUT:
                # head for the final 4 steps
                emit_head4(nc, php, pfp, ewp, fc1_sb, fc1b_sb, fc2w_sb,
                           hT8, out_acc, t_steps - 4)

            # bulk store of the accumulated head partials (token-major)
            nc.sync.dma_start(
                out_part[:].rearrange("(b t) o -> b t o", t=t_steps),
                out_acc[:])

            # ---- tail: ReduceScatter fc2 partials, bias + log_softmax ----
            rs_out = dramp1.tile([n_tok_loc, O], F32)
            nc.gpsimd.collective_compute(
                "ReduceScatter",
                ALU.add,
                replica_groups=[list(range(N_CORES))],
                ins=[out_part[:].opt()],
                outs=[rs_out[:].opt()],
            )
            # token rows viewed as [128, CH, O]: partition p owns tokens
            # p*CH..(p+1)*CH-1 — contiguous per-partition DMA lines
            CH = n_tok_loc // 128
            z_sb = ewp.tile([128, CH, O], F32)
            nc.sync.dma_start(
                z_sb[:], rs_out[:].rearrange("(p c) o -> p c o", p=128))
            res_sb = ewp.tile([128, CH, O], F32)
            for ch in range(CH):
                zb = ewp.tile([128, O], F32)
                nc.vector.tensor_add(zb[:], z_sb[:, ch, :], fc2b_sb)
                mx = ewp.tile([128, 1], F32)
                nc.vector.reduce_max(mx[:], zb[:], axis=mybir.AxisListType.X)
                sh = ewp.tile([128, O], F32)
                nc.vector.tensor_scalar_sub(sh[:], zb[:], mx[:])
                ex = ewp.tile([128, O], F32)
                nc.scalar.activation(ex[:], sh[:], AF.Exp)
                sm = ewp.tile([128, 1], F32)
                nc.vector.reduce_sum(sm[:], ex[:], axis=mybir.AxisListType.X)
                lg = ewp.tile([128, 1], F32)
                nc.scalar.activation(lg[:], sm[:], AF.Ln)
                nc.vector.tensor_scalar_sub(res_sb[:, ch, :], sh[:], lg[:])
            nc.sync.dma_start(
                out_sh[:].rearrange("(p c) o -> p c o", p=128), res_sb[:])

    nc.compile()
    return nc


def emit_head4(nc, php, pfp, ewp, fc1_sb, fc1b_sb, fc2w_sb, hT8, out_acc, t0):
    """fc1 + relu + fc2 partials for steps t0..t0+3 (slots contiguous in the
    8-ring because t0 % 4 == 0)."""
    s0 = t0 % 8
    ph = php.tile([128, 4 * B], F32)
    for kc in range(NKH):
        nc.tensor.matmul(ph[:], fc1_sb[:, kc, :], hT8[:, s0:s0 + 4, kc, :],
                         start=(kc == 0), stop=(kc == NKH - 1))
    hid = ewp.tile([128, 4, B], BF16)
    nc.scalar.activation(hid[:], ph[:].rearrange("p (q b) -> p q b", q=4),
                         AF.Relu, bias=fc1b_sb)
    pf = pfp.tile([128, 4 * O], F32)
    for s in range(4):
        nc.tensor.matmul(pf[:, s * O:(s + 1) * O], hid[:, s, :], fc2w_sb,
                         start=True, stop=True)
    nc.vector.tensor_copy(
        out_acc[:, t0:t0 + 4, :],
        pf[:].rearrange("p (q o) -> p q o", q=4))


_CACHED = {}


def _get_kernel(t_steps):
    if t_steps not in _CACHED:
        _CACHED[t_steps] = build_kernel(t_steps)
    return _CACHED[t_steps]


def prep_inputs(m_text, m_audio, m_video, W_ih, W_hh, b_ih, b_hh,
                fc1_w, fc1_b, fc2_w, fc2_b, t_steps=T):
    """Host-side layout prep; returns per-core input maps."""
    bf = ml_dtypes.bfloat16
    x = np.concatenate([np.asarray(m_text), np.asarray(m_audio),
                        np.asarray(m_video)], axis=-1).astype(np.float32)
    b_, t_, d_ = x.shape
    assert (b_, d_) == (B, D) and t_ == t_steps
    # x_T: [T, D+1(ones) padded to 4*128, B], stored [t4, p, k, t', b] so the
    # precompute load is one contiguous 2KB line per partition
    xTf = np.zeros((t_steps, NKX * 128, B), np.float32)
    xTf[:, :D, :] = x.transpose(1, 2, 0)
    xTf[:, D, :] = 1.0
    xTf = np.ascontiguousarray(
        xTf.reshape(t_steps // 4, 4, NKX, 128, B).transpose(
            0, 3, 2, 1, 4)).astype(ml_dtypes.float8_e4m3)

    W_ih = np.asarray(W_ih, np.float32)
    W_hh = np.asarray(W_hh, np.float32)
    bias = (np.asarray(b_ih) + np.asarray(b_hh)).astype(np.float32)
    fc1_w = np.asarray(fc1_w, np.float32)
    fc1_b = np.asarray(fc1_b, np.float32)
    fc2_w = np.asarray(fc2_w, np.float32)
    fc2_b = np.asarray(fc2_b, np.float32)

    in_maps = []
    gate_order = (0, 1, 2, 3)  # i, f, g, o (pytorch row-block order)
    eye = np.eye(128, dtype=np.float32)
    for k in range(N_CORES):
        rows = np.concatenate(
            [np.arange(g * H + k * HSL, g * H + (k + 1) * HSL)
             for g in gate_order])
        wih_sl = W_ih[rows, :]            # [512, 409]
        whh_sl = W_hh[rows, :]            # [512, 1024]
        b_sl = bias[rows]                 # [512]
        wihT_k = np.zeros((NKX * 128, GSL), np.float32)
        wihT_k[:D, :] = wih_sl.T
        wihT_k[D, :] = b_sl
        # p-major: [128, NKX*GSL]
        wih_pm = wihT_k.reshape(NKX, 128, GSL).transpose(1, 0, 2).reshape(
            128, NKX * GSL)
        whh_pm = whh_sl.T.reshape(NKH, 128, GSL).transpose(1, 0, 2).reshape(
            128, NKH * GSL)

        cc = k % 4                        # C chunk (cores k and k+4 duplicate)
        crows = np.arange(cc * 128, (cc + 1) * 128)
        fc1_pm = fc1_w[crows, :].T.reshape(NKH, 128, 128).transpose(
            1, 0, 2).reshape(128, NKH * 128)
        fc2wT_k = 0.5 * fc2_w[:, crows].T                    # [128, O]
        wblob_k = np.ascontiguousarray(np.concatenate(
            [wih_pm, whh_pm, fc1_pm, fc2wT_k, eye], axis=1)).astype(bf)

        fc1b_k = fc1_b[crows].reshape(128, 1)
        fc2b_k = np.broadcast_to(fc2_b[None, :], (128, O))
        fblob_k = np.ascontiguousarray(np.concatenate(
            [fc1b_k, fc2b_k], axis=1)).astype(np.float32)

        t_sh4 = t_steps // N_CORES // 4
        in_maps.append({
            "xTs": xTf[k * t_sh4:(k + 1) * t_sh4],
            "wblob": wblob_k,
            "fblob": fblob_k,
        })
    return in_maps


def run(inputs, t_steps=T, trace=False):
    nc = _get_kernel(t_steps)
    in_maps = prep_inputs(
        inputs["m_text"], inputs["m_audio"], inputs["m_video"],
        inputs["W_ih"], inputs["W_hh"], inputs["b_ih"], inputs["b_hh"],
        inputs["fc1_w"], inputs["fc1_b"], inputs["fc2_w"], inputs["fc2_b"],
        t_steps=t_steps)
    res = run_bass_kernel_spmd(
        nc, in_maps, core_ids=list(range(N_CORES)), trace=trace)
    shards = [res.results[k]["out_sh"] for k in range(N_CORES)]
    full = np.concatenate(shards, axis=0)          # [(b t), O] token-major
    out = full.reshape(B, t_steps, O)
    return out, res


def kernel(**inputs) -> np.ndarray:
    t_steps = np.asarray(inputs["m_text"]).shape[1]
    out, _ = run(inputs, t_steps=t_steps)
    return out.astype(np.float32)
